# revision 23
# baseline (speedup 1.0000x reference)
"""IsoGMM loss kernel for 8 Trainium2 NeuronCores.

loss = mean_{n,k} r[n,k] * ||X[n] - mus[k]||^2

Decomposition (everything folds into PE-accumulated panels per core):
  sum_{n,k} r*d2 = T1 + T2 - 2*T3
    T1 = sum_{k,d} A[k,d],   A = r.T @ (X*X)
    T2 = sum_k musq_k * C_k, C_k = sum_n r[n,k]
    T3 = sum_{k,d} mus[k,d] * M[k,d],  M = r.T @ X

Per 128-row segment the PE accumulates two panels from the same r weights:
  psA[64,129] += r_seg.T @ [X_seg | 1]   (col 128 -> C_k, ones shipped in X)
  psB[64,128] += r_seg.T @ Z_seg,  Z = X*X
so no DVE row-reduction is ever needed (TensorReduce has no 2x mode and
would bottleneck). The final [64,257] panel is combined on host in fp64.

Precision: X is cast to bf16 (keeps the DVE square in 2x perf mode -
fp8 operands drop DVE to 1x, which would be the bottleneck), r to fp8e4
(weights side only); Z is bf16; PSUM accumulates fp32. Validated rel
err ~6e-5 vs the 2e-2 gate. The kernel is DMA-bound end to end.

Pipeline: graduated chunk sizes (small first chunks so the PE starts
~4us earlier; small last chunks so the tail is tight), every tile is
resident for the whole kernel (no buffer-recycle stalls), X streams on
the sync-engine DGE queue while r streams on the scalar-engine queue.
GpSimd is kept fully idle: its software ops are ~3x slower than DVE and
its activity lowers the chip power-throttle budget for everyone else.
"""

import numpy as np
import ml_dtypes

import concourse.bass as bass
import concourse.mybir as mybir
import concourse.tile as tile
from concourse import bacc
from concourse.bass_utils import run_bass_kernel_spmd

N, K, D = 131072, 64, 128
NCORES = 8
NS = N // NCORES       # rows per core
RPP = NS // 128        # rows per SBUF partition (= segments per core)
XCH = (4, 8, 12, 16, 16, 16, 16, 16, 16, 8)   # segments per X chunk (sum = RPP)
RCH = (16, 48, 64)                   # segments per r chunk
WX = D + 1             # X panel width: 128 cols + host-side ones column
WOUT = 2 * D + 1       # output panel width: M | C | A
GP_SPLIT = 0.0         # GpSimd square share (0: GpSimd stays idle)


def build_nc():
    assert sum(XCH) == RPP and sum(RCH) == RPP
    f32 = mybir.dt.float32
    bf16 = mybir.dt.bfloat16
    f8 = mybir.dt.float8e4

    # Bacc (not plain Bass): its compile() splits sync waits to satisfy
    # TRN2's 1-wait-per-instruction limit, which walrus enforces.
    nc = bacc.Bacc("TRN2", target_bir_lowering=False, debug=False)
    xp = nc.dram_tensor("xp", [128, RPP * WX], bf16, kind="ExternalInput")
    rp = nc.dram_tensor("rp", [128, RPP * K], f8, kind="ExternalInput")
    out = nc.dram_tensor("out", [K, WOUT], f32, kind="ExternalOutput")

    rbounds = np.cumsum(RCH)  # segment index -> r chunk via searchsorted

    with (
        tile.TileContext(nc) as tc,
        tc.tile_pool(name="xb", bufs=len(XCH)) as xpool,
        tc.tile_pool(name="rb", bufs=len(RCH)) as rpool,
        tc.tile_pool(name="zb", bufs=len(XCH)) as zpool,
        tc.tile_pool(name="one", bufs=1) as onepool,
        tc.tile_pool(name="ps", bufs=1, space="PSUM") as pspool,
    ):
        psA = pspool.tile([K, WX], f32)
        psB = pspool.tile([K, D], f32)

        # r chunks stream on the scalar-engine DGE queue, in parallel with
        # the X issues on the sync queue.
        rtiles = []
        roff = 0
        for rs in RCH:
            rt = rpool.tile([128, rs * K], f8, tag="r")
            nc.scalar.dma_start(out=rt, in_=rp[:, roff * K:(roff + rs) * K])
            rtiles.append(rt.rearrange("p (s k) -> p s k", k=K))
            roff += rs

        def lhsT(s):
            ri = int(np.searchsorted(rbounds, s, side="right"))
            jr = s - (int(rbounds[ri - 1]) if ri else 0)
            return rtiles[ri][:, jr, :]

        s0 = 0
        for spc in XCH:
            xt = xpool.tile([128, spc * WX], bf16, tag="x")
            nc.sync.dma_start(out=xt, in_=xp[:, s0 * WX:(s0 + spc) * WX])
            x3 = xt.rearrange("p (s w) -> p s w", w=WX)

            # Z = X*X -> bf16 (DVE TENSOR_TENSOR in 2x perf mode).
            zt = zpool.tile([128, spc * D], bf16, tag="z")
            z3 = zt.rearrange("p (s d) -> p s d", d=D)
            nc.vector.tensor_mul(z3, x3[:, :, 0:D], x3[:, :, 0:D])

            last = s0 + spc == RPP
            if not last:
                for j in range(spc):
                    s = s0 + j
                    nc.tensor.matmul(
                        psA, lhsT=lhsT(s), rhs=x3[:, j, :],
                        start=(s == 0), stop=False,
                    )
                    nc.tensor.matmul(
                        psB, lhsT=lhsT(s), rhs=z3[:, j, :],
                        start=(s == 0), stop=False,
                    )
            else:
                # Close psA first: its PSUM->SBUF copy and output DMA then
                # overlap the last chunk's psB matmuls.
                for ps, src3 in ((psA, x3), (psB, z3)):
                    for j in range(spc):
                        nc.tensor.matmul(
                            ps, lhsT=lhsT(s0 + j), rhs=src3[:, j, :],
                            start=False, stop=(j == spc - 1),
                        )
            s0 += spc

        # Ship both accumulated panels; final weighted sum happens on host.
        # Two copies + two DMAs so the first panel ships while the second
        # is still being copied out of PSUM.
        osb = onepool.tile([K, WOUT], f32)
        nc.vector.tensor_copy(osb[:, 0:WX], psA)
        nc.sync.dma_start(out=out[:, 0:WX], in_=osb[:, 0:WX])
        nc.vector.tensor_copy(osb[:, WX:WOUT], psB)
        nc.scalar.dma_start(out=out[:, WX:WOUT], in_=osb[:, WX:WOUT])

    nc.compile()
    return nc


def make_in_maps(X, r, mus, ncores=NCORES):
    X = np.asarray(X, dtype=np.float32)
    r = np.asarray(r, dtype=np.float32)
    n = X.shape[0]
    ns = n // ncores

    Xa = np.empty((n, WX), dtype=ml_dtypes.bfloat16)
    Xa[:, 0:D] = X.astype(ml_dtypes.bfloat16)
    Xa[:, D] = 1.0
    rq = r.astype(ml_dtypes.float8_e4m3fn)

    in_maps = []
    for i in range(ncores):
        in_maps.append(
            {
                "xp": np.ascontiguousarray(
                    Xa[i * ns:(i + 1) * ns].reshape(128, (ns // 128) * WX)
                ),
                "rp": np.ascontiguousarray(
                    rq[i * ns:(i + 1) * ns].reshape(128, (ns // 128) * K)
                ),
            }
        )
    return in_maps


def combine_outputs(results, mus):
    """Unshard: weighted sum of each core's [K, 2D+1] panel -> mean.

    panel[:, 0:128]   = M = r.T @ X       -> T3 via <mus, M>
    panel[:, 128]     = C_k = sum_n r     -> T2 via <musq, C>
    panel[:, 129:257] = A = r.T @ (X*X)   -> T1 via sum(A)
    """
    mus = np.asarray(mus, dtype=np.float64)
    musq = (mus ** 2).sum(1)
    total = 0.0
    for res in results:
        p = res["out"].astype(np.float64)
        M = p[:, 0:D]
        C = p[:, D]
        A = p[:, WX:WOUT]
        total += A.sum() + musq @ C - 2.0 * (mus * M).sum()
    return np.array(total / (N * K), dtype=np.float32)


def kernel(X, r, mus):
    nc = build_nc()
    in_maps = make_in_maps(X, r, mus)
    res = run_bass_kernel_spmd(nc, in_maps, list(range(NCORES)))
    return combine_outputs(res.results[:NCORES], mus)


# revision 24
# speedup vs baseline: 1.1553x; 1.1553x over previous
"""IsoGMM loss kernel for 8 Trainium2 NeuronCores.

loss = mean_{n,k} r[n,k] * ||X[n] - mus[k]||^2

Decomposition (everything folds into PE-accumulated panels per core):
  sum_{n,k} r*d2 = T1 + T2 - 2*T3
    T1 = sum_{k,d} A[k,d],   A = r.T @ (X*X)
    T2 = sum_k musq_k * C_k, C_k = sum_n r[n,k]
    T3 = sum_{k,d} mus[k,d] * M[k,d],  M = r.T @ X

Per 128-row segment the PE accumulates two panels from the same r weights:
  psA[64,129] += r_seg.T @ [X_seg | 1]   (col 128 -> C_k, ones shipped in X)
  psB[64,128] += r_seg.T @ Z_seg,  Z = X*X
so no DVE row-reduction is ever needed (TensorReduce has no 2x mode and
would bottleneck). The final [64,257] panel is combined on host in fp64.

Precision: X is cast to bf16 (keeps the DVE square in 2x perf mode -
fp8 operands drop DVE to 1x, which would be the bottleneck), r to fp8e4
(weights side only); Z is bf16; PSUM accumulates fp32. Validated rel
err ~6e-5 vs the 2e-2 gate. The kernel is DMA-bound end to end.

Pipeline: graduated chunk sizes (small first chunks so the PE starts
~4us earlier; small last chunks so the tail is tight), every tile is
resident for the whole kernel (no buffer-recycle stalls), X streams on
the sync-engine DGE queue while r streams on the scalar-engine queue.
GpSimd is kept fully idle: its software ops are ~3x slower than DVE and
its activity lowers the chip power-throttle budget for everyone else.
"""

import numpy as np
import ml_dtypes

import concourse.bass as bass
import concourse.mybir as mybir
import concourse.tile as tile
from concourse import bacc
from concourse.bass_utils import run_bass_kernel_spmd

N, K, D = 131072, 64, 128
NCORES = 8
NS = N // NCORES       # rows per core
RPP = NS // 128        # rows per SBUF partition (= segments per core)
XCH = (4, 8, 12, 16, 16, 16, 16, 16, 16, 8)   # segments per X chunk (sum = RPP)
RCH = (16, 48, 64)                   # segments per r chunk
WX = D + 1             # X panel width: 128 cols + host-side ones column
WOUT = 2 * D + 1       # output panel width: M | C | A
GP_SPLIT = 0.0         # GpSimd square share (0: GpSimd stays idle)


def build_nc():
    assert sum(XCH) == RPP and sum(RCH) == RPP
    f32 = mybir.dt.float32
    bf16 = mybir.dt.bfloat16
    f8 = mybir.dt.float8e4

    # Bacc (not plain Bass): its compile() splits sync waits to satisfy
    # TRN2's 1-wait-per-instruction limit, which walrus enforces.
    nc = bacc.Bacc("TRN2", target_bir_lowering=False, debug=False)
    xp = nc.dram_tensor("xp", [128, RPP * WX], bf16, kind="ExternalInput")
    rp = nc.dram_tensor("rp", [128, RPP * K], f8, kind="ExternalInput")
    out = nc.dram_tensor("out", [K, WOUT], f32, kind="ExternalOutput")

    rbounds = np.cumsum(RCH)  # segment index -> r chunk via searchsorted

    with (
        tile.TileContext(nc) as tc,
        tc.tile_pool(name="xb", bufs=len(XCH)) as xpool,
        tc.tile_pool(name="rb", bufs=len(RCH)) as rpool,
        tc.tile_pool(name="zb", bufs=len(XCH)) as zpool,
        tc.tile_pool(name="one", bufs=1) as onepool,
        tc.tile_pool(name="ps", bufs=1, space="PSUM") as pspool,
    ):
        psA = pspool.tile([K, WX], f32)
        psB = pspool.tile([K, D], f32)

        # r chunks stream on the scalar-engine DGE queue, in parallel with
        # the X issues on the sync queue.
        rtiles = []
        roff = 0
        for rs in RCH:
            rt = rpool.tile([128, rs * K], f8, tag="r")
            nc.scalar.dma_start(out=rt, in_=rp[:, roff * K:(roff + rs) * K])
            rtiles.append(rt.rearrange("p (s k) -> p s k", k=K))
            roff += rs

        def lhsT(s):
            ri = int(np.searchsorted(rbounds, s, side="right"))
            jr = s - (int(rbounds[ri - 1]) if ri else 0)
            return rtiles[ri][:, jr, :]

        s0 = 0
        for spc in XCH:
            xt = xpool.tile([128, spc * WX], bf16, tag="x")
            nc.sync.dma_start(out=xt, in_=xp[:, s0 * WX:(s0 + spc) * WX])
            x3 = xt.rearrange("p (s w) -> p s w", w=WX)

            # Z = X*X -> bf16 (DVE TENSOR_TENSOR in 2x perf mode).
            zt = zpool.tile([128, spc * D], bf16, tag="z")
            z3 = zt.rearrange("p (s d) -> p s d", d=D)
            nc.vector.tensor_mul(z3, x3[:, :, 0:D], x3[:, :, 0:D])

            last = s0 + spc == RPP
            if not last:
                for j in range(spc):
                    s = s0 + j
                    nc.tensor.matmul(
                        psA, lhsT=lhsT(s), rhs=x3[:, j, :],
                        start=(s == 0), stop=False,
                    )
                    nc.tensor.matmul(
                        psB, lhsT=lhsT(s), rhs=z3[:, j, :],
                        start=(s == 0), stop=False,
                    )
            else:
                # Close psA first: its PSUM->SBUF copy and output DMA then
                # overlap the last chunk's psB matmuls.
                for ps, src3 in ((psA, x3), (psB, z3)):
                    for j in range(spc):
                        nc.tensor.matmul(
                            ps, lhsT=lhsT(s0 + j), rhs=src3[:, j, :],
                            start=False, stop=(j == spc - 1),
                        )
            s0 += spc

        # Trailing dummy read: the queue's final transfer trickles in
        # noticeably slower (last ~150KB in every trace); this sacrificial
        # read absorbs that so the real last chunk completes at full rate.
        scr = onepool.tile([128, 768], bf16)
        nc.sync.dma_start(out=scr, in_=xp[:, 0:768])

        # Ship both accumulated panels; final weighted sum happens on host.
        # Two copies + two DMAs so the first panel ships while the second
        # is still being copied out of PSUM.
        osb = onepool.tile([K, WOUT], f32)
        nc.vector.tensor_copy(osb[:, 0:WX], psA)
        nc.sync.dma_start(out=out[:, 0:WX], in_=osb[:, 0:WX])
        nc.vector.tensor_copy(osb[:, WX:WOUT], psB)
        nc.scalar.dma_start(out=out[:, WX:WOUT], in_=osb[:, WX:WOUT])

    nc.compile()
    return nc


def make_in_maps(X, r, mus, ncores=NCORES):
    X = np.asarray(X, dtype=np.float32)
    r = np.asarray(r, dtype=np.float32)
    n = X.shape[0]
    ns = n // ncores

    Xa = np.empty((n, WX), dtype=ml_dtypes.bfloat16)
    Xa[:, 0:D] = X.astype(ml_dtypes.bfloat16)
    Xa[:, D] = 1.0
    rq = r.astype(ml_dtypes.float8_e4m3fn)

    in_maps = []
    for i in range(ncores):
        in_maps.append(
            {
                "xp": np.ascontiguousarray(
                    Xa[i * ns:(i + 1) * ns].reshape(128, (ns // 128) * WX)
                ),
                "rp": np.ascontiguousarray(
                    rq[i * ns:(i + 1) * ns].reshape(128, (ns // 128) * K)
                ),
            }
        )
    return in_maps


def combine_outputs(results, mus):
    """Unshard: weighted sum of each core's [K, 2D+1] panel -> mean.

    panel[:, 0:128]   = M = r.T @ X       -> T3 via <mus, M>
    panel[:, 128]     = C_k = sum_n r     -> T2 via <musq, C>
    panel[:, 129:257] = A = r.T @ (X*X)   -> T1 via sum(A)
    """
    mus = np.asarray(mus, dtype=np.float64)
    musq = (mus ** 2).sum(1)
    total = 0.0
    for res in results:
        p = res["out"].astype(np.float64)
        M = p[:, 0:D]
        C = p[:, D]
        A = p[:, WX:WOUT]
        total += A.sum() + musq @ C - 2.0 * (mus * M).sum()
    return np.array(total / (N * K), dtype=np.float32)


def kernel(X, r, mus):
    nc = build_nc()
    in_maps = make_in_maps(X, r, mus)
    res = run_bass_kernel_spmd(nc, in_maps, list(range(NCORES)))
    return combine_outputs(res.results[:NCORES], mus)


# revision 25
# speedup vs baseline: 1.1593x; 1.0035x over previous
"""IsoGMM loss kernel for 8 Trainium2 NeuronCores.

loss = mean_{n,k} r[n,k] * ||X[n] - mus[k]||^2

Decomposition (everything folds into PE-accumulated panels per core):
  sum_{n,k} r*d2 = T1 + T2 - 2*T3
    T1 = sum_{k,d} A[k,d],   A = r.T @ (X*X)
    T2 = sum_k musq_k * C_k, C_k = sum_n r[n,k]
    T3 = sum_{k,d} mus[k,d] * M[k,d],  M = r.T @ X

Per 128-row segment the PE accumulates two panels from the same r weights:
  psA[64,129] += r_seg.T @ [X_seg | 1]   (col 128 -> C_k, ones shipped in X)
  psB[64,128] += r_seg.T @ Z_seg,  Z = X*X
so no DVE row-reduction is ever needed (TensorReduce has no 2x mode and
would bottleneck). The final [64,257] panel is combined on host in fp64.

Precision: X is cast to bf16 (keeps the DVE square in 2x perf mode -
fp8 operands drop DVE to 1x, which would be the bottleneck), r to fp8e4
(weights side only); Z is bf16; PSUM accumulates fp32. Validated rel
err ~6e-5 vs the 2e-2 gate. The kernel is DMA-bound end to end.

Pipeline: graduated chunk sizes (small first chunks so the PE starts
~4us earlier; small last chunks so the tail is tight), every tile is
resident for the whole kernel (no buffer-recycle stalls), X streams on
the sync-engine DGE queue while r streams on the scalar-engine queue.
GpSimd is kept fully idle: its software ops are ~3x slower than DVE and
its activity lowers the chip power-throttle budget for everyone else.
"""

import numpy as np
import ml_dtypes

import concourse.bass as bass
import concourse.mybir as mybir
import concourse.tile as tile
from concourse import bacc
from concourse.bass_utils import run_bass_kernel_spmd

N, K, D = 131072, 64, 128
NCORES = 8
NS = N // NCORES       # rows per core
RPP = NS // 128        # rows per SBUF partition (= segments per core)
XCH = (4, 8, 12, 16, 16, 16, 16, 16, 16, 8)   # segments per X chunk (sum = RPP)
RCH = (16, 48, 64)                   # segments per r chunk
WX = D + 1             # X panel width: 128 cols + host-side ones column
WOUT = 2 * D + 1       # output panel width: M | C | A
GP_SPLIT = 0.0         # GpSimd square share (0: GpSimd stays idle)


def build_nc():
    assert sum(XCH) == RPP and sum(RCH) == RPP
    f32 = mybir.dt.float32
    bf16 = mybir.dt.bfloat16
    f8 = mybir.dt.float8e4

    # Bacc (not plain Bass): its compile() splits sync waits to satisfy
    # TRN2's 1-wait-per-instruction limit, which walrus enforces.
    nc = bacc.Bacc("TRN2", target_bir_lowering=False, debug=False)
    xp = nc.dram_tensor("xp", [128, RPP * WX], bf16, kind="ExternalInput")
    rp = nc.dram_tensor("rp", [128, RPP * K], f8, kind="ExternalInput")
    out = nc.dram_tensor("out", [K, WOUT], f32, kind="ExternalOutput")

    rbounds = np.cumsum(RCH)  # segment index -> r chunk via searchsorted

    with (
        tile.TileContext(nc) as tc,
        tc.tile_pool(name="xb", bufs=len(XCH)) as xpool,
        tc.tile_pool(name="rb", bufs=len(RCH)) as rpool,
        tc.tile_pool(name="zb", bufs=len(XCH)) as zpool,
        tc.tile_pool(name="one", bufs=1) as onepool,
        tc.tile_pool(name="ps", bufs=1, space="PSUM") as pspool,
    ):
        psA = pspool.tile([K, WX], f32)
        psB = pspool.tile([K, D], f32)

        # r chunks stream on the scalar-engine DGE queue, in parallel with
        # the X issues on the sync queue.
        rtiles = []
        roff = 0
        for rs in RCH:
            rt = rpool.tile([128, rs * K], f8, tag="r")
            nc.scalar.dma_start(out=rt, in_=rp[:, roff * K:(roff + rs) * K])
            rtiles.append(rt.rearrange("p (s k) -> p s k", k=K))
            roff += rs

        def lhsT(s):
            ri = int(np.searchsorted(rbounds, s, side="right"))
            jr = s - (int(rbounds[ri - 1]) if ri else 0)
            return rtiles[ri][:, jr, :]

        s0 = 0
        for spc in XCH:
            xt = xpool.tile([128, spc * WX], bf16, tag="x")
            nc.sync.dma_start(out=xt, in_=xp[:, s0 * WX:(s0 + spc) * WX])
            x3 = xt.rearrange("p (s w) -> p s w", w=WX)

            # Z = X*X -> bf16 (DVE TENSOR_TENSOR in 2x perf mode).
            zt = zpool.tile([128, spc * D], bf16, tag="z")
            z3 = zt.rearrange("p (s d) -> p s d", d=D)
            nc.vector.tensor_mul(z3, x3[:, :, 0:D], x3[:, :, 0:D])

            last = s0 + spc == RPP
            if not last:
                for j in range(spc):
                    s = s0 + j
                    nc.tensor.matmul(
                        psA, lhsT=lhsT(s), rhs=x3[:, j, :],
                        start=(s == 0), stop=False,
                    )
                    nc.tensor.matmul(
                        psB, lhsT=lhsT(s), rhs=z3[:, j, :],
                        start=(s == 0), stop=False,
                    )
            else:
                # Close psA first: its PSUM->SBUF copy and output DMA then
                # overlap the last chunk's psB matmuls.
                for ps, src3 in ((psA, x3), (psB, z3)):
                    for j in range(spc):
                        nc.tensor.matmul(
                            ps, lhsT=lhsT(s0 + j), rhs=src3[:, j, :],
                            start=False, stop=(j == spc - 1),
                        )
            s0 += spc

        # Trailing dummy reads: a queue's final transfer trickles in
        # noticeably slower (last ~150KB in every trace), delaying its
        # completion semaphore. A sacrificial read at the end of each DGE
        # queue absorbs that, so the last real chunk on the sync queue and
        # r2 on the scalar queue (gating all segments >= 64) complete at
        # full rate. Nothing consumes either read.
        scr = onepool.tile([128, 768], bf16)
        nc.sync.dma_start(out=scr, in_=xp[:, 0:768])
        scr2 = onepool.tile([128, 1024], f8)
        nc.scalar.dma_start(out=scr2, in_=rp[:, 0:1024])

        # Ship both accumulated panels; final weighted sum happens on host.
        # Two copies + two DMAs so the first panel ships while the second
        # is still being copied out of PSUM.
        osb = onepool.tile([K, WOUT], f32)
        nc.vector.tensor_copy(osb[:, 0:WX], psA)
        nc.sync.dma_start(out=out[:, 0:WX], in_=osb[:, 0:WX])
        nc.vector.tensor_copy(osb[:, WX:WOUT], psB)
        nc.scalar.dma_start(out=out[:, WX:WOUT], in_=osb[:, WX:WOUT])

    nc.compile()
    return nc


def make_in_maps(X, r, mus, ncores=NCORES):
    X = np.asarray(X, dtype=np.float32)
    r = np.asarray(r, dtype=np.float32)
    n = X.shape[0]
    ns = n // ncores

    Xa = np.empty((n, WX), dtype=ml_dtypes.bfloat16)
    Xa[:, 0:D] = X.astype(ml_dtypes.bfloat16)
    Xa[:, D] = 1.0
    rq = r.astype(ml_dtypes.float8_e4m3fn)

    in_maps = []
    for i in range(ncores):
        in_maps.append(
            {
                "xp": np.ascontiguousarray(
                    Xa[i * ns:(i + 1) * ns].reshape(128, (ns // 128) * WX)
                ),
                "rp": np.ascontiguousarray(
                    rq[i * ns:(i + 1) * ns].reshape(128, (ns // 128) * K)
                ),
            }
        )
    return in_maps


def combine_outputs(results, mus):
    """Unshard: weighted sum of each core's [K, 2D+1] panel -> mean.

    panel[:, 0:128]   = M = r.T @ X       -> T3 via <mus, M>
    panel[:, 128]     = C_k = sum_n r     -> T2 via <musq, C>
    panel[:, 129:257] = A = r.T @ (X*X)   -> T1 via sum(A)
    """
    mus = np.asarray(mus, dtype=np.float64)
    musq = (mus ** 2).sum(1)
    total = 0.0
    for res in results:
        p = res["out"].astype(np.float64)
        M = p[:, 0:D]
        C = p[:, D]
        A = p[:, WX:WOUT]
        total += A.sum() + musq @ C - 2.0 * (mus * M).sum()
    return np.array(total / (N * K), dtype=np.float32)


def kernel(X, r, mus):
    nc = build_nc()
    in_maps = make_in_maps(X, r, mus)
    res = run_bass_kernel_spmd(nc, in_maps, list(range(NCORES)))
    return combine_outputs(res.results[:NCORES], mus)
